# revision 1
# baseline (speedup 1.0000x reference)
"""Grouped SwiGLU MoE MLP (16 experts) on 8 NeuronCores, expert-parallel.

Reference computation, per expert e over its contiguous token slice xi:
    out = (silu(xi @ w_gate[e].T) * (xi @ w_up[e].T)) @ w_down[e].T

Sharding: expert-parallel. Core c owns experts {2c, 2c+1}; the host hands it
the matching contiguous 2048-token slice of x (tokens are pre-sorted by
expert), so no device-side collectives are needed. Everything is handed to
the device feature-major (transposed on host) so the token axis is the
matmul moving/free dimension:

  gateT[f,t] = sum_h wgT[h,f] * xT[h,t]      (PE: lhsT=wgT tile, rhs=xT)
  hidT[f,t]  = silu(gateT) * upT             (ACT silu + DVE mul)
  outT[h,t]  = sum_f wdT[f,h] * hidT[f,t]    (PE: lhsT=wdT tile, rhs=hidT)

float32r matmuls (full PE rate at N>=512 free dim), fp32 PSUM accumulation.
Weights stream through SBUF in >=1 MiB chunks; x and hidden stay resident.
"""

import numpy as np

import concourse.bass as bass
import concourse.bacc as bacc
import concourse.mybir as mybir
from concourse import tile
from concourse.bass_utils import run_bass_kernel_spmd

E, T, H, F = 16, 16384, 1024, 2048
NCORES = 8
EPC = E // NCORES          # experts per core
TPE = T // E               # tokens per expert (uniform fast path)
P = 128                    # SBUF partitions
HT = H // P                # 8 h-tiles (contraction tiles for gate/up)
FT = F // P                # 16 f-tiles
HGS = H // P               # 8 output h-groups for down proj
NT = 512                   # matmul moving free dim (PSUM bank = 512 fp32)
TH = TPE // NT             # 2 t-halves
FG = 8                     # f-groups for gate/up weight streaming
FPG = FT // FG             # f-tiles per group = 2
FGW = F // FG              # f columns per group = 256

_F32 = mybir.dt.float32
_F32R = mybir.dt.float32r

_CACHE = {}

# Set by run for test harness introspection (exec_time_ns, profile).
LAST_RESULTS = None
TRACE = False
TRACE_KW = {}
# "silu" uses the native ScalarE Silu LUT; "sigmoid" decomposes it as
# gate*sigmoid(gate) for CoreSim, which lacks a Silu implementation.
ACT_MODE = "silu"


def _build_nc():
    nc = bacc.Bacc()
    xt_d = nc.dram_tensor("xt", [EPC, H, TPE], _F32R, kind="ExternalInput")
    wg_d = nc.dram_tensor("wg", [EPC, H, F], _F32R, kind="ExternalInput")
    wu_d = nc.dram_tensor("wu", [EPC, H, F], _F32R, kind="ExternalInput")
    wd_d = nc.dram_tensor("wd", [EPC, F, H], _F32R, kind="ExternalInput")
    out_d = nc.dram_tensor("outT", [EPC, H, TPE], _F32, kind="ExternalOutput")

    with tile.TileContext(nc) as tc:
        with (
            tc.tile_pool(name="xp", bufs=8) as xp,
            tc.tile_pool(name="wgp", bufs=3) as wgp,
            tc.tile_pool(name="wup", bufs=3) as wup,
            tc.tile_pool(name="wdp", bufs=3) as wdp,
            tc.tile_pool(name="hid", bufs=FT + 1) as hidp,
            tc.tile_pool(name="tmp", bufs=3) as tmpp,
            tc.tile_pool(name="osb", bufs=3) as osbp,
            tc.tile_pool(name="ps", bufs=8, space=bass.MemorySpace.PSUM) as psp,
        ):
            for el in range(EPC):
                # DRAM views with the h-tile index split out of the partition
                # axis: [128p, HT, F].
                wg_v = wg_d[el].rearrange("(a p) f -> p a f", p=P)
                wu_v = wu_d[el].rearrange("(a p) f -> p a f", p=P)

                # fg0's weight chunks go out ahead of the bulk x load so the
                # first matmul chain can start as soon as x's first h-tile
                # lands instead of behind the whole 4 MiB of x.
                fsl0 = slice(0, FGW)
                wgt0 = wgp.tile([P, HT, FGW], _F32R, tag="wg")
                nc.sync.dma_start(wgt0[:], wg_v[:, :, fsl0])
                wut0 = wup.tile([P, HT, FGW], _F32R, tag="wu")
                nc.sync.dma_start(wut0[:], wu_v[:, :, fsl0])

                # Token activations, resident for the whole expert: 8 tiles
                # [128h, 1024t], alternated across the two HWDGE rings so the
                # startup fill isn't serialized on one ring's FIFO.
                xts = []
                for ht in range(HT):
                    xt = xp.tile([P, TPE], _F32R, tag="xt")
                    dma_eng = nc.sync if ht % 2 == 0 else nc.scalar
                    dma_eng.dma_start(xt[:], xt_d[el, ht * P:(ht + 1) * P, :])
                    xts.append(xt)

                hidden = [hidp.tile([P, TPE], _F32R, tag="hid", name=f"hid{el}_{i}") for i in range(FT)]

                for fgi in range(FG):
                    if fgi == 0:
                        wgt, wut = wgt0, wut0
                    else:
                        fsl = slice(fgi * FGW, (fgi + 1) * FGW)
                        wgt = wgp.tile([P, HT, FGW], _F32R, tag="wg")
                        nc.sync.dma_start(wgt[:], wg_v[:, :, fsl])
                        wut = wup.tile([P, HT, FGW], _F32R, tag="wu")
                        nc.sync.dma_start(wut[:], wu_v[:, :, fsl])

                    gate_ps, up_ps = {}, {}
                    for wt, store in ((wgt, gate_ps), (wut, up_ps)):
                        for ftl in range(FPG):
                            for th in range(TH):
                                store[ftl, th] = psp.tile([P, NT], _F32, tag="ps", name="gu_ps")
                            for ht in range(HT):
                                lhsT = wt[:, ht, ftl * P:(ftl + 1) * P]
                                for th in range(TH):
                                    nc.tensor.matmul(
                                        store[ftl, th][:],
                                        lhsT,
                                        xts[ht][:, th * NT:(th + 1) * NT],
                                        start=(ht == 0),
                                        stop=(ht == HT - 1),
                                    )
                    for ftl in range(FPG):
                        ft = fgi * FPG + ftl
                        for th in range(TH):
                            tsl = slice(th * NT, (th + 1) * NT)
                            tmp = tmpp.tile([P, NT], _F32, tag="tmp")
                            if ACT_MODE == "silu":
                                nc.scalar.activation(
                                    tmp[:], gate_ps[ftl, th][:],
                                    mybir.ActivationFunctionType.Silu,
                                )
                            else:
                                nc.scalar.activation(
                                    tmp[:], gate_ps[ftl, th][:],
                                    mybir.ActivationFunctionType.Sigmoid,
                                )
                                nc.vector.tensor_mul(
                                    tmp[:], tmp[:], gate_ps[ftl, th][:]
                                )
                            nc.vector.tensor_mul(
                                hidden[ft][:, tsl], tmp[:], up_ps[ftl, th][:]
                            )

                # Down projection: outT[h,t] accumulating over all 16 f-tiles.
                wd_v = wd_d[el].rearrange("(a p) h -> p a h", p=P)
                for hg in range(HGS):
                    hsl = slice(hg * P, (hg + 1) * P)
                    wdt = wdp.tile([P, FT, P], _F32R, tag="wd")
                    nc.sync.dma_start(wdt[:], wd_v[:, :, hsl])
                    ops = [psp.tile([P, NT], _F32, tag="ps", name="dn_ps") for _ in range(TH)]
                    for ft in range(FT):
                        lhsT = wdt[:, ft, :]
                        for th in range(TH):
                            nc.tensor.matmul(
                                ops[th][:],
                                lhsT,
                                hidden[ft][:, th * NT:(th + 1) * NT],
                                start=(ft == 0),
                                stop=(ft == FT - 1),
                            )
                    osb = osbp.tile([P, TPE], _F32, tag="osb")
                    for th in range(TH):
                        nc.vector.tensor_copy(osb[:, th * NT:(th + 1) * NT], ops[th][:])
                    # Stores go out on the ACT HWDGE ring so they never queue
                    # behind pending weight loads on the SP ring.
                    nc.scalar.dma_start(out_d[el, hsl, :], osb[:])
    return nc


def get_nc():
    if "nc" not in _CACHE:
        nc = _build_nc()
        nc.finalize()
        _CACHE["nc"] = nc
    return _CACHE["nc"]


def make_in_maps(x, w_gate, w_up, w_down):
    in_maps = []
    for c in range(NCORES):
        e0 = c * EPC
        xs = x[e0 * TPE:(e0 + EPC) * TPE].reshape(EPC, TPE, H)
        in_maps.append({
            "xt": np.ascontiguousarray(xs.transpose(0, 2, 1)),
            "wg": np.ascontiguousarray(w_gate[e0:e0 + EPC].transpose(0, 2, 1)),
            "wu": np.ascontiguousarray(w_up[e0:e0 + EPC].transpose(0, 2, 1)),
            "wd": np.ascontiguousarray(w_down[e0:e0 + EPC].transpose(0, 2, 1)),
        })
    return in_maps


def _numpy_fallback(x, w_gate, w_up, w_down, counts):
    out = np.empty((x.shape[0], w_down.shape[1]), np.float32)
    o = 0
    for e in range(len(counts)):
        n = int(counts[e])
        xi = x[o:o + n]
        gate = xi @ w_gate[e].T
        up = xi @ w_up[e].T
        hidden = (gate / (1.0 + np.exp(-gate))) * up
        out[o:o + n] = hidden @ w_down[e].T
        o += n
    return out


def kernel(x, w_gate, w_up, w_down, tokens_per_expert):
    global LAST_RESULTS
    x = np.asarray(x, dtype=np.float32)
    w_gate = np.asarray(w_gate, dtype=np.float32)
    w_up = np.asarray(w_up, dtype=np.float32)
    w_down = np.asarray(w_down, dtype=np.float32)
    counts = np.asarray(tokens_per_expert).astype(np.int64)

    if not (counts.shape == (E,) and np.all(counts == TPE)):
        # Non-uniform routing: the compiled program is shaped for the
        # uniform split the reference generator produces.
        return _numpy_fallback(x, w_gate, w_up, w_down, counts)

    nc = get_nc()
    res = run_bass_kernel_spmd(
        nc, make_in_maps(x, w_gate, w_up, w_down), list(range(NCORES)),
        trace=TRACE, **TRACE_KW,
    )
    LAST_RESULTS = res
    out = np.empty((T, H), np.float32)
    for c in range(NCORES):
        o = res.results[c]["outT"]  # [EPC, H, TPE]
        for el in range(EPC):
            t0 = (c * EPC + el) * TPE
            out[t0:t0 + TPE] = o[el].T
    return out



# revision 4
# speedup vs baseline: 1.2474x; 1.2474x over previous
"""Grouped SwiGLU MoE MLP (16 experts) on 8 NeuronCores, expert-parallel.

Reference computation, per expert e over its contiguous token slice xi:
    out = (silu(xi @ w_gate[e].T) * (xi @ w_up[e].T)) @ w_down[e].T

Sharding: expert-parallel. Core c owns experts {2c, 2c+1}; the host hands it
the matching contiguous 2048-token slice of x (tokens are pre-sorted by
expert), so no device-side collectives are needed. Everything is handed to
the device feature-major (transposed on host) so the token axis is the
matmul moving/free dimension:

  gateT[f,t] = sum_h wgT[h,f] * xT[h,t]      (PE: lhsT=wgT tile, rhs=xT)
  hidT[f,t]  = silu(gateT) * upT             (ACT silu + DVE mul)
  outT[h,t]  = sum_f wdT[f,h] * hidT[f,t]    (PE: lhsT=wdT tile, rhs=hidT)

Inputs are quantized to bf16 on the host (rel err ~4e-3, budget 2e-2):
bf16 matmuls run at the same 1 row/cycle PE rate as fp32r but get the
fast-weight-load path (fp32 LDWEIGHTS at 224ns/128cols outruns the 213ns
matmul and throttles the pipeline), and halve DMA + SBUF footprint.
fp32 PSUM accumulation, fp32 output.  x for both experts loads upfront on
the ACT HWDGE ring; weights stream on the SP ring; stores go on ACT.
"""

import numpy as np
import ml_dtypes

import concourse.bass as bass
import concourse.bacc as bacc
import concourse.mybir as mybir
from concourse import tile
from concourse.bass_utils import run_bass_kernel_spmd

E, T, H, F = 16, 16384, 1024, 2048
NCORES = 8
EPC = E // NCORES          # experts per core
TPE = T // E               # tokens per expert (uniform fast path)
P = 128                    # SBUF partitions
HT = H // P                # 8 h-tiles (contraction tiles for gate/up)
FT = F // P                # 16 f-tiles
HGS = H // P               # 8 output h-groups for down proj
NT = 512                   # matmul moving free dim (PSUM bank = 512 fp32)
TH = TPE // NT             # 2 t-halves
FG = 8                     # f-groups for gate/up weight streaming
FPG = FT // FG             # f-tiles per group = 2
FGW = F // FG              # f columns per group = 256
HP = 4                     # down-proj h-group pairs (2 hg per weight chunk)

_F32 = mybir.dt.float32
_BF16 = mybir.dt.bfloat16
_NPBF16 = ml_dtypes.bfloat16

_CACHE = {}

# Set by run for test harness introspection (exec_time_ns, profile).
LAST_RESULTS = None
TRACE = False
TRACE_KW = {}
# "silu" uses the native ScalarE Silu LUT; "sigmoid" decomposes it as
# gate*sigmoid(gate) for CoreSim, which lacks a Silu implementation.
ACT_MODE = "silu"


def _build_nc():
    nc = bacc.Bacc()
    xt_d = nc.dram_tensor("xt", [EPC, H, TPE], _BF16, kind="ExternalInput")
    wg_d = nc.dram_tensor("wg", [EPC, H, F], _BF16, kind="ExternalInput")
    wu_d = nc.dram_tensor("wu", [EPC, H, F], _BF16, kind="ExternalInput")
    wd_d = nc.dram_tensor("wd", [EPC, F, H], _BF16, kind="ExternalInput")
    out_d = nc.dram_tensor("outT", [EPC, H, TPE], _F32, kind="ExternalOutput")

    with tile.TileContext(nc) as tc:
        with (
            tc.tile_pool(name="xp", bufs=2 * HT) as xp,
            tc.tile_pool(name="wgp", bufs=4) as wgp,
            tc.tile_pool(name="wup", bufs=4) as wup,
            tc.tile_pool(name="wdp", bufs=3) as wdp,
            tc.tile_pool(name="hid", bufs=FT + 1) as hidp,
            tc.tile_pool(name="tmp", bufs=3) as tmpp,
            tc.tile_pool(name="osb", bufs=4) as osbp,
            tc.tile_pool(name="ps", bufs=8, space=bass.MemorySpace.PSUM) as psp,
        ):
            # Token activations for BOTH experts, loaded upfront on the ACT
            # HWDGE ring (weights own the SP ring).  Resident all kernel:
            # 16 tiles x [128h, 1024t] bf16 = 32 KiB/partition.
            xts_all = []
            for el in range(EPC):
                xts = []
                for ht in range(HT):
                    xt = xp.tile([P, TPE], _BF16, tag="xt")
                    nc.scalar.dma_start(xt[:], xt_d[el, ht * P:(ht + 1) * P, :])
                    xts.append(xt)
                xts_all.append(xts)

            for el in range(EPC):
                xts = xts_all[el]
                # DRAM views with the h-tile index split out of the partition
                # axis: [128p, HT, F].
                wg_v = wg_d[el].rearrange("(a p) f -> p a f", p=P)
                wu_v = wu_d[el].rearrange("(a p) f -> p a f", p=P)

                hidden = [hidp.tile([P, TPE], _BF16, tag="hid",
                                    name=f"hid{el}_{i}") for i in range(FT)]

                for fgi in range(FG):
                    fsl = slice(fgi * FGW, (fgi + 1) * FGW)
                    if el == 0 and fgi == 0:
                        # First weight chunk split so the opening matmul only
                        # waits on the ht0-1 slice (128 KiB), not the full
                        # 512 KiB chunk.
                        wga = wgp.tile([P, 2, FGW], _BF16, tag="wg")
                        nc.sync.dma_start(wga[:], wg_v[:, 0:2, fsl])
                        wgb = wgp.tile([P, HT - 2, FGW], _BF16, tag="wg")
                        nc.sync.dma_start(wgb[:], wg_v[:, 2:HT, fsl])
                        wg_sl = lambda ht, c0, c1, a=wga, b=wgb: (
                            a[:, ht, c0:c1] if ht < 2 else b[:, ht - 2, c0:c1])
                        wut = wup.tile([P, HT, FGW], _BF16, tag="wu")
                        nc.sync.dma_start(wut[:], wu_v[:, :, fsl])
                        wu_sl = lambda ht, c0, c1, w=wut: w[:, ht, c0:c1]
                    else:
                        wgt = wgp.tile([P, HT, FGW], _BF16, tag="wg")
                        nc.sync.dma_start(wgt[:], wg_v[:, :, fsl])
                        wut = wup.tile([P, HT, FGW], _BF16, tag="wu")
                        nc.sync.dma_start(wut[:], wu_v[:, :, fsl])
                        wg_sl = lambda ht, c0, c1, w=wgt: w[:, ht, c0:c1]
                        wu_sl = lambda ht, c0, c1, w=wut: w[:, ht, c0:c1]

                    gate_ps, up_ps = {}, {}
                    for wsl, store in ((wg_sl, gate_ps), (wu_sl, up_ps)):
                        for ftl in range(FPG):
                            for th in range(TH):
                                store[ftl, th] = psp.tile(
                                    [P, NT], _F32, tag="ps", name="gu_ps")
                            for ht in range(HT):
                                lhsT = wsl(ht, ftl * P, (ftl + 1) * P)
                                for th in range(TH):
                                    nc.tensor.matmul(
                                        store[ftl, th][:],
                                        lhsT,
                                        xts[ht][:, th * NT:(th + 1) * NT],
                                        start=(ht == 0),
                                        stop=(ht == HT - 1),
                                    )
                    for ftl in range(FPG):
                        ft = fgi * FPG + ftl
                        for th in range(TH):
                            tsl = slice(th * NT, (th + 1) * NT)
                            tmp = tmpp.tile([P, NT], _F32, tag="tmp")
                            if ACT_MODE == "silu":
                                nc.scalar.activation(
                                    tmp[:], gate_ps[ftl, th][:],
                                    mybir.ActivationFunctionType.Silu,
                                )
                            else:
                                nc.scalar.activation(
                                    tmp[:], gate_ps[ftl, th][:],
                                    mybir.ActivationFunctionType.Sigmoid,
                                )
                                nc.vector.tensor_mul(
                                    tmp[:], tmp[:], gate_ps[ftl, th][:]
                                )
                            nc.vector.tensor_mul(
                                hidden[ft][:, tsl], tmp[:], up_ps[ftl, th][:]
                            )

                # Down projection: outT[h,t] accumulating over all 16 f-tiles.
                # Weights come in hg-pairs ([128, 16, 256] bf16) so DRAM
                # segments stay at 512B.
                wd_v = wd_d[el].rearrange("(a p) h -> p a h", p=P)
                for hp in range(HP):
                    wdt = wdp.tile([P, FT, 2 * P], _BF16, tag="wd")
                    nc.sync.dma_start(
                        wdt[:], wd_v[:, :, hp * 2 * P:(hp + 1) * 2 * P])
                    for sub in range(2):
                        hg = 2 * hp + sub
                        hsl = slice(hg * P, (hg + 1) * P)
                        ops = [psp.tile([P, NT], _F32, tag="ps", name="dn_ps")
                               for _ in range(TH)]
                        for ft in range(FT):
                            lhsT = wdt[:, ft, sub * P:(sub + 1) * P]
                            for th in range(TH):
                                nc.tensor.matmul(
                                    ops[th][:],
                                    lhsT,
                                    hidden[ft][:, th * NT:(th + 1) * NT],
                                    start=(ft == 0),
                                    stop=(ft == FT - 1),
                                )
                        # Per-th copies + stores so the final transfer after
                        # the last matmul is only 256 KiB.  Stores ride the
                        # ACT ring; x loads there finished long ago.
                        for th in range(TH):
                            tsl = slice(th * NT, (th + 1) * NT)
                            osb = osbp.tile([P, NT], _F32, tag="osb")
                            nc.vector.tensor_copy(osb[:], ops[th][:])
                            nc.scalar.dma_start(out_d[el, hsl, tsl], osb[:])
    return nc


def get_nc():
    if "nc" not in _CACHE:
        nc = _build_nc()
        nc.finalize()
        _CACHE["nc"] = nc
    return _CACHE["nc"]


def make_in_maps(x, w_gate, w_up, w_down):
    in_maps = []
    for c in range(NCORES):
        e0 = c * EPC
        xs = x[e0 * TPE:(e0 + EPC) * TPE].reshape(EPC, TPE, H)
        in_maps.append({
            "xt": np.ascontiguousarray(
                xs.transpose(0, 2, 1)).astype(_NPBF16),
            "wg": np.ascontiguousarray(
                w_gate[e0:e0 + EPC].transpose(0, 2, 1)).astype(_NPBF16),
            "wu": np.ascontiguousarray(
                w_up[e0:e0 + EPC].transpose(0, 2, 1)).astype(_NPBF16),
            "wd": np.ascontiguousarray(
                w_down[e0:e0 + EPC].transpose(0, 2, 1)).astype(_NPBF16),
        })
    return in_maps


def _numpy_fallback(x, w_gate, w_up, w_down, counts):
    out = np.empty((x.shape[0], w_down.shape[1]), np.float32)
    o = 0
    for e in range(len(counts)):
        n = int(counts[e])
        xi = x[o:o + n]
        gate = xi @ w_gate[e].T
        up = xi @ w_up[e].T
        hidden = (gate / (1.0 + np.exp(-gate))) * up
        out[o:o + n] = hidden @ w_down[e].T
        o += n
    return out


def kernel(x, w_gate, w_up, w_down, tokens_per_expert):
    global LAST_RESULTS
    x = np.asarray(x, dtype=np.float32)
    w_gate = np.asarray(w_gate, dtype=np.float32)
    w_up = np.asarray(w_up, dtype=np.float32)
    w_down = np.asarray(w_down, dtype=np.float32)
    counts = np.asarray(tokens_per_expert).astype(np.int64)

    if not (counts.shape == (E,) and np.all(counts == TPE)):
        # Non-uniform routing: the compiled program is shaped for the
        # uniform split the reference generator produces.
        return _numpy_fallback(x, w_gate, w_up, w_down, counts)

    nc = get_nc()
    res = run_bass_kernel_spmd(
        nc, make_in_maps(x, w_gate, w_up, w_down), list(range(NCORES)),
        trace=TRACE, **TRACE_KW,
    )
    LAST_RESULTS = res
    out = np.empty((T, H), np.float32)
    for c in range(NCORES):
        o = res.results[c]["outT"]  # [EPC, H, TPE]
        for el in range(EPC):
            t0 = (c * EPC + el) * TPE
            out[t0:t0 + TPE] = o[el].T
    return out


# revision 8
# speedup vs baseline: 1.2580x; 1.0085x over previous
"""Grouped SwiGLU MoE MLP (16 experts) on 8 NeuronCores, expert-parallel.

Reference computation, per expert e over its contiguous token slice xi:
    out = (silu(xi @ w_gate[e].T) * (xi @ w_up[e].T)) @ w_down[e].T

Sharding: expert-parallel. Core c owns experts {2c, 2c+1}; the host hands it
the matching contiguous 2048-token slice of x (tokens are pre-sorted by
expert), so no device-side collectives are needed. Everything is handed to
the device feature-major (transposed on host) so the token axis is the
matmul moving/free dimension:

  gateT[f,t] = sum_h wgT[h,f] * xT[h,t]      (PE: lhsT=wgT tile, rhs=xT)
  hidT[f,t]  = silu(gateT) * upT             (ACT silu + DVE mul)
  outT[h,t]  = sum_f wdT[f,h] * hidT[f,t]    (PE: lhsT=wdT tile, rhs=hidT)

Inputs are quantized to bf16 on the host (rel err ~4e-3, budget 2e-2):
bf16 matmuls run at the same 1 row/cycle PE rate as fp32r but get the
fast-weight-load path (fp32 LDWEIGHTS at 224ns/128cols outruns the 213ns
matmul and throttles the pipeline), and halve DMA + SBUF footprint.
fp32 PSUM accumulation, fp32 output.  x for both experts loads upfront on
the ACT HWDGE ring; weights stream on the SP ring; stores go on ACT.
"""

import numpy as np
import ml_dtypes

import concourse.bass as bass
import concourse.bacc as bacc
import concourse.mybir as mybir
from concourse import tile
from concourse.bass_utils import run_bass_kernel_spmd

E, T, H, F = 16, 16384, 1024, 2048
NCORES = 8
EPC = E // NCORES          # experts per core
TPE = T // E               # tokens per expert (uniform fast path)
P = 128                    # SBUF partitions
HT = H // P                # 8 h-tiles (contraction tiles for gate/up)
FT = F // P                # 16 f-tiles
HGS = H // P               # 8 output h-groups for down proj
NT = 512                   # matmul moving free dim (PSUM bank = 512 fp32)
TH = TPE // NT             # 2 t-halves
FG = 8                     # f-groups for gate/up weight streaming
FPG = FT // FG             # f-tiles per group = 2
FGW = F // FG              # f columns per group = 256
HP = 4                     # down-proj h-group pairs (2 hg per weight chunk)

_F32 = mybir.dt.float32
_BF16 = mybir.dt.bfloat16
_NPBF16 = ml_dtypes.bfloat16

_CACHE = {}

# Set by run for test harness introspection (exec_time_ns, profile).
LAST_RESULTS = None
TRACE = False
TRACE_KW = {}
# "silu" uses the native ScalarE Silu LUT; "sigmoid" decomposes it as
# gate*sigmoid(gate) for CoreSim, which lacks a Silu implementation.
ACT_MODE = "silu"


def _build_nc():
    nc = bacc.Bacc()
    xt_d = nc.dram_tensor("xt", [EPC, H, TPE], _BF16, kind="ExternalInput")
    wg_d = nc.dram_tensor("wg", [EPC, H, F], _BF16, kind="ExternalInput")
    wu_d = nc.dram_tensor("wu", [EPC, H, F], _BF16, kind="ExternalInput")
    wd_d = nc.dram_tensor("wd", [EPC, F, H], _BF16, kind="ExternalInput")
    out_d = nc.dram_tensor("outT", [EPC, H, TPE], _F32, kind="ExternalOutput")

    with tile.TileContext(nc) as tc:
        with (
            tc.tile_pool(name="xp", bufs=2 * HT) as xp,
            tc.tile_pool(name="wgp", bufs=4) as wgp,
            tc.tile_pool(name="wup", bufs=4) as wup,
            tc.tile_pool(name="wdp", bufs=3) as wdp,
            tc.tile_pool(name="hid", bufs=FT + 1) as hidp,
            tc.tile_pool(name="tmp", bufs=3) as tmpp,
            tc.tile_pool(name="osb", bufs=4) as osbp,
            tc.tile_pool(name="ps", bufs=8, space=bass.MemorySpace.PSUM) as psp,
        ):
            # PE warmup: a few matmuls on zeroed tiles issued while the first
            # real DMAs are in flight, so the HAM clock-gate opens (1.2 ->
            # 2.4 GHz takes ~3.4us of sustained PE activity) before real
            # matmuls start.
            wl = tmpp.tile([P, P], _BF16, tag="warml")
            wr = tmpp.tile([P, NT], _BF16, tag="warmr")
            nc.gpsimd.memset(wl[:], 0)
            nc.gpsimd.memset(wr[:], 0)
            wps = psp.tile([P, NT], _F32, tag="ps", name="warm_ps")
            for _ in range(4):
                nc.tensor.matmul(wps[:], wl[:], wr[:], start=True, stop=True)

            # Startup-critical loads, ordered so both HWDGE rings fill the
            # first f-group's operands in parallel with the x stream:
            #   SP ring:  wg[ht0-1] -> wg[ht2-7] -> wu[ht4-7] -> (groups...)
            #   ACT ring: x[e0,ht0] -> wu[ht0-3] -> x rest (both experts)
            # so all 8 PSUM chains of group 0 can start as soon as each x
            # h-tile lands.
            wg_v0 = wg_d[0].rearrange("(a p) f -> p a f", p=P)
            wu_v0 = wu_d[0].rearrange("(a p) f -> p a f", p=P)
            fsl0 = slice(0, FGW)
            wga = wgp.tile([P, 2, FGW], _BF16, tag="wg")
            nc.sync.dma_start(wga[:], wg_v0[:, 0:2, fsl0])
            wgb = wgp.tile([P, HT - 2, FGW], _BF16, tag="wg")
            nc.sync.dma_start(wgb[:], wg_v0[:, 2:HT, fsl0])
            wub = wup.tile([P, HT - 4, FGW], _BF16, tag="wu")
            nc.sync.dma_start(wub[:], wu_v0[:, 4:HT, fsl0])

            # Token activations for BOTH experts on the ACT HWDGE ring.
            # Resident all kernel: 16 tiles x [128h, 1024t] bf16 =
            # 32 KiB/partition.
            xts_all = [[], []]
            xt00 = xp.tile([P, TPE], _BF16, tag="xt")
            nc.scalar.dma_start(xt00[:], xt_d[0, 0:P, :])
            xts_all[0].append(xt00)
            wua = wup.tile([P, 4, FGW], _BF16, tag="wu")
            nc.scalar.dma_start(wua[:], wu_v0[:, 0:4, fsl0])
            for el in range(EPC):
                for ht in range(HT):
                    if el == 0 and ht == 0:
                        continue
                    xt = xp.tile([P, TPE], _BF16, tag="xt")
                    nc.scalar.dma_start(xt[:], xt_d[el, ht * P:(ht + 1) * P, :])
                    xts_all[el].append(xt)

            for el in range(EPC):
                xts = xts_all[el]
                # DRAM views with the h-tile index split out of the partition
                # axis: [128p, HT, F].
                wg_v = wg_d[el].rearrange("(a p) f -> p a f", p=P)
                wu_v = wu_d[el].rearrange("(a p) f -> p a f", p=P)

                hidden = [hidp.tile([P, TPE], _BF16, tag="hid",
                                    name=f"hid{el}_{i}") for i in range(FT)]

                for fgi in range(FG):
                    fsl = slice(fgi * FGW, (fgi + 1) * FGW)
                    if el == 0 and fgi == 0:
                        # Uses the startup-split tiles loaded above.
                        wg_sl = lambda ht, c0, c1, a=wga, b=wgb: (
                            a[:, ht, c0:c1] if ht < 2 else b[:, ht - 2, c0:c1])
                        wu_sl = lambda ht, c0, c1, a=wua, b=wub: (
                            a[:, ht, c0:c1] if ht < 4 else b[:, ht - 4, c0:c1])
                    else:
                        wgt = wgp.tile([P, HT, FGW], _BF16, tag="wg")
                        nc.sync.dma_start(wgt[:], wg_v[:, :, fsl])
                        wut = wup.tile([P, HT, FGW], _BF16, tag="wu")
                        nc.sync.dma_start(wut[:], wu_v[:, :, fsl])
                        wg_sl = lambda ht, c0, c1, w=wgt: w[:, ht, c0:c1]
                        wu_sl = lambda ht, c0, c1, w=wut: w[:, ht, c0:c1]

                    gate_ps, up_ps = {}, {}
                    for wsl, store in ((wg_sl, gate_ps), (wu_sl, up_ps)):
                        for ftl in range(FPG):
                            for th in range(TH):
                                store[ftl, th] = psp.tile(
                                    [P, NT], _F32, tag="ps", name="gu_ps")
                            for ht in range(HT):
                                lhsT = wsl(ht, ftl * P, (ftl + 1) * P)
                                for th in range(TH):
                                    nc.tensor.matmul(
                                        store[ftl, th][:],
                                        lhsT,
                                        xts[ht][:, th * NT:(th + 1) * NT],
                                        start=(ht == 0),
                                        stop=(ht == HT - 1),
                                    )
                    for ftl in range(FPG):
                        ft = fgi * FPG + ftl
                        for th in range(TH):
                            tsl = slice(th * NT, (th + 1) * NT)
                            tmp = tmpp.tile([P, NT], _F32, tag="tmp")
                            if ACT_MODE == "silu":
                                nc.scalar.activation(
                                    tmp[:], gate_ps[ftl, th][:],
                                    mybir.ActivationFunctionType.Silu,
                                )
                            else:
                                nc.scalar.activation(
                                    tmp[:], gate_ps[ftl, th][:],
                                    mybir.ActivationFunctionType.Sigmoid,
                                )
                                nc.vector.tensor_mul(
                                    tmp[:], tmp[:], gate_ps[ftl, th][:]
                                )
                            nc.vector.tensor_mul(
                                hidden[ft][:, tsl], tmp[:], up_ps[ftl, th][:]
                            )

                # Down projection: outT[h,t] accumulating over all 16 f-tiles.
                # Weights come in hg-pairs ([128, 16, 256] bf16) so DRAM
                # segments stay at 512B.
                wd_v = wd_d[el].rearrange("(a p) h -> p a h", p=P)
                for hp in range(HP):
                    wdt = wdp.tile([P, FT, 2 * P], _BF16, tag="wd")
                    nc.sync.dma_start(
                        wdt[:], wd_v[:, :, hp * 2 * P:(hp + 1) * 2 * P])
                    for sub in range(2):
                        hg = 2 * hp + sub
                        hsl = slice(hg * P, (hg + 1) * P)
                        ops = [psp.tile([P, NT], _F32, tag="ps", name="dn_ps")
                               for _ in range(TH)]
                        for ft in range(FT):
                            lhsT = wdt[:, ft, sub * P:(sub + 1) * P]
                            for th in range(TH):
                                nc.tensor.matmul(
                                    ops[th][:],
                                    lhsT,
                                    hidden[ft][:, th * NT:(th + 1) * NT],
                                    start=(ft == 0),
                                    stop=(ft == FT - 1),
                                )
                        # Per-th copies + stores so the final transfer after
                        # the last matmul is only 256 KiB.  Stores ride the
                        # ACT ring; x loads there finished long ago.  The
                        # very last output is further quartered so the
                        # kernel-tail copy+store pipeline is as short as
                        # possible.
                        last = (el == EPC - 1 and hp == HP - 1 and sub == 1)
                        for th in range(TH):
                            nsl = 4 if (last and th == TH - 1) else 1
                            step = NT // nsl
                            for s in range(nsl):
                                c0 = th * NT + s * step
                                tsl = slice(c0, c0 + step)
                                osb = osbp.tile([P, step], _F32, tag="osb")
                                nc.vector.tensor_copy(
                                    osb[:], ops[th][:, s * step:(s + 1) * step])
                                nc.scalar.dma_start(
                                    out_d[el, hsl, tsl], osb[:])
    return nc


def get_nc():
    if "nc" not in _CACHE:
        nc = _build_nc()
        nc.finalize()
        _CACHE["nc"] = nc
    return _CACHE["nc"]


def make_in_maps(x, w_gate, w_up, w_down):
    in_maps = []
    for c in range(NCORES):
        e0 = c * EPC
        xs = x[e0 * TPE:(e0 + EPC) * TPE].reshape(EPC, TPE, H)
        in_maps.append({
            "xt": np.ascontiguousarray(
                xs.transpose(0, 2, 1)).astype(_NPBF16),
            "wg": np.ascontiguousarray(
                w_gate[e0:e0 + EPC].transpose(0, 2, 1)).astype(_NPBF16),
            "wu": np.ascontiguousarray(
                w_up[e0:e0 + EPC].transpose(0, 2, 1)).astype(_NPBF16),
            "wd": np.ascontiguousarray(
                w_down[e0:e0 + EPC].transpose(0, 2, 1)).astype(_NPBF16),
        })
    return in_maps


def _numpy_fallback(x, w_gate, w_up, w_down, counts):
    out = np.empty((x.shape[0], w_down.shape[1]), np.float32)
    o = 0
    for e in range(len(counts)):
        n = int(counts[e])
        xi = x[o:o + n]
        gate = xi @ w_gate[e].T
        up = xi @ w_up[e].T
        hidden = (gate / (1.0 + np.exp(-gate))) * up
        out[o:o + n] = hidden @ w_down[e].T
        o += n
    return out


def kernel(x, w_gate, w_up, w_down, tokens_per_expert):
    global LAST_RESULTS
    x = np.asarray(x, dtype=np.float32)
    w_gate = np.asarray(w_gate, dtype=np.float32)
    w_up = np.asarray(w_up, dtype=np.float32)
    w_down = np.asarray(w_down, dtype=np.float32)
    counts = np.asarray(tokens_per_expert).astype(np.int64)

    if not (counts.shape == (E,) and np.all(counts == TPE)):
        # Non-uniform routing: the compiled program is shaped for the
        # uniform split the reference generator produces.
        return _numpy_fallback(x, w_gate, w_up, w_down, counts)

    nc = get_nc()
    res = run_bass_kernel_spmd(
        nc, make_in_maps(x, w_gate, w_up, w_down), list(range(NCORES)),
        trace=TRACE, **TRACE_KW,
    )
    LAST_RESULTS = res
    out = np.empty((T, H), np.float32)
    for c in range(NCORES):
        o = res.results[c]["outT"]  # [EPC, H, TPE]
        for el in range(EPC):
            t0 = (c * EPC + el) * TPE
            out[t0:t0 + TPE] = o[el].T
    return out
